# revision 9
# baseline (speedup 1.0000x reference)
"""Trainium2 Bass kernel for nn_BasisAlignment (loss_fn).

Math: the reference's [N,N] distance map factorizes as dm[i,j] = m1[i] - m2[j]
(m = row means). Its masked min/max over j are order selections:
  l_dist[i] = fl32(m1[i] - A[i]),  A[i] = max{ m2[j] : fl32(m1[i]-m2[j]) >  EPS }
  g_dist[i] = fl32(m1[i] - B[i]),  B[i] = min{ m2[j] : fl32(m1[i]-m2[j]) < -EPS }
so it suffices to know, per query m1[i], how many m2[j] lie strictly below it
(a rank count), plus the sorted m2 values.

Device (SPMD over 8 NeuronCores, feats row-sharded 512 rows/core):
  - row means of the local feats1/feats2 blocks: two sequential 512-wide DVE
    reduces + add + *2^-10 (bit-identical to jnp.mean's neuron lowering —
    verified on hardware)
  - AllGather of the m2 partial means -> full m2 [4096] on every core
  - rank counts: per local m1 query row, sum_j (m2[j] < m1[i]) via fused
    is_lt+accumulate DVE instructions (the hint's "local masked min/max", in
    count form; integer-valued fp32, exact, order-invariant). Computed in 4
    quarter passes pipelined behind partition-broadcast fills of m2.
Host: sort m2, convert ranks to neighbor values with an exact fp32 boundary
correction for the +-EPS band, then run the reference's tiny [N] epilogue with
the same eager jnp ops (their divide rounding is backend-specific and the
loss is at rounding-noise scale, so the epilogue must use identical ops).
"""
import numpy as np

N = 4096
D = 1024
NCORES = 8
RPC = N // NCORES        # rows per core
BPC = RPC // 128         # 128-row blocks per core
NQ = 4                   # rank quarter passes
NCOLS = 2 * BPC + NQ * BPC

BIG = 1.0e12
EPS = 1.0e-12

_compiled = None


def _build():
    import concourse.bacc as bacc
    import concourse.mybir as mybir
    from concourse import tile

    F32 = mybir.dt.float32
    AX = mybir.AxisListType
    OP = mybir.AluOpType

    nc = bacc.Bacc("TRN2", target_bir_lowering=False, debug=False,
                   num_devices=NCORES)
    f1 = nc.declare_dram_parameter("f1", [RPC, D], F32, isOutput=False)
    f2 = nc.declare_dram_parameter("f2", [RPC, D], F32, isOutput=False)
    # out columns: 0:4 m1 means, 4:8 m2 means, 8:24 rank quarter-counts
    # (col group 2+q, lane b): global row = core*512 + b*128 + partition
    out = nc.declare_dram_parameter("out", [128, NCOLS], F32, isOutput=True)

    with tile.TileContext(nc) as tc:
        with (
            tc.tile_pool(name="data", bufs=4 * BPC) as data_pool,
            tc.tile_pool(name="stage", bufs=1) as stage_pool,
            tc.tile_pool(name="wide", bufs=1) as wide_pool,
            tc.tile_pool(name="scr", bufs=2) as scr_pool,
            tc.tile_pool(name="dram", bufs=1, space="DRAM") as dram_pool,
        ):
            stage = stage_pool.tile([128, NCOLS], F32)
            m2_bounce = dram_pool.tile([128, BPC], F32)
            m2_all = dram_pool.tile([NCORES * 128, BPC], F32)

            def means(src, col0):
                # per 128-row block: half-tiles loaded separately so the
                # first reduce starts sooner; the reduce/add/scale structure
                # is bit-identical to jnp.mean's neuron lowering
                for b in range(BPC):
                    t0 = data_pool.tile([128, D // 2], F32, tag="th")
                    t1 = data_pool.tile([128, D // 2], F32, tag="th")
                    nc.sync.dma_start(t0[:], src[b * 128:(b + 1) * 128, 0:D // 2])
                    nc.sync.dma_start(t1[:], src[b * 128:(b + 1) * 128, D // 2:D])
                    r0 = scr_pool.tile([128, 1], F32, tag="r0")
                    r1 = scr_pool.tile([128, 1], F32, tag="r1")
                    nc.vector.tensor_reduce(r0[:], t0[:], axis=AX.X, op=OP.add)
                    nc.vector.tensor_reduce(r1[:], t1[:], axis=AX.X, op=OP.add)
                    h = scr_pool.tile([128, 1], F32, tag="h")
                    nc.vector.tensor_tensor(h[:], r0[:], r1[:], op=OP.add)
                    nc.vector.tensor_scalar_mul(
                        stage[:, col0 + b:col0 + b + 1], h[:], float(2.0 ** -10))

            # feats2 means first so the collective starts early
            means(f2, BPC)
            nc.sync.dma_start(m2_bounce[:, :], stage[:, BPC:2 * BPC])
            nc.gpsimd.collective_compute(
                "AllGather", OP.bypass,
                replica_groups=[list(range(NCORES))],
                ins=[m2_bounce.opt()], outs=[m2_all.opt()],
            )

            # feats1 means overlap with the collective
            means(f1, 0)

            # m2 broadcast to all partitions, in quarters pipelined against
            # the rank passes (counts are order-invariant, so the gathered
            # permutation of m2 is irrelevant)
            m2row = wide_pool.tile([1, N], F32, tag="m2row")
            nc.sync.dma_start(m2row[:], m2_all.rearrange("a b -> (a b)"))
            m2b = wide_pool.tile([128, N], F32, tag="m2b")
            H = N // NQ
            for q in range(NQ):
                nc.gpsimd.partition_broadcast(
                    m2b[:, q * H:(q + 1) * H], m2row[:, q * H:(q + 1) * H])
                for b in range(BPC):
                    junk = scr_pool.tile([128, H], F32, tag="junk")
                    nc.vector.tensor_scalar(
                        out=junk[:], in0=m2b[:, q * H:(q + 1) * H],
                        scalar1=stage[:, b:b + 1], scalar2=0.0,
                        op0=OP.is_lt, op1=OP.add,
                        accum_out=stage[:, (2 + q) * BPC + b:(2 + q) * BPC + b + 1],
                    )

            nc.sync.dma_start(out[:, :], stage[:])

    nc.compile()
    return nc


def _get_compiled():
    global _compiled
    if _compiled is None:
        _compiled = _build()
    return _compiled


def _run_device(feats1, feats2, trace=False):
    from concourse.bass_utils import run_bass_kernel_spmd

    nc = _get_compiled()
    in_maps = []
    for c in range(NCORES):
        in_maps.append({
            "f1": np.ascontiguousarray(feats1[c * RPC:(c + 1) * RPC], dtype=np.float32),
            "f2": np.ascontiguousarray(feats2[c * RPC:(c + 1) * RPC], dtype=np.float32),
        })
    res = run_bass_kernel_spmd(nc, in_maps, list(range(NCORES)), trace=trace)
    m1 = np.empty(N, np.float32)
    m2 = np.empty(N, np.float32)
    rank = np.zeros(N, np.float64)
    for c in range(NCORES):
        o = res.results[c]["out"]  # [128, NCOLS]
        for b in range(BPC):
            sl = slice(c * RPC + b * 128, c * RPC + (b + 1) * 128)
            m1[sl] = o[:, b]
            m2[sl] = o[:, BPC + b]
            acc = np.zeros(128, np.float64)
            for q in range(NQ):
                acc += o[:, (2 + q) * BPC + b].astype(np.float64)
            rank[sl] = acc
    return m1, m2, rank, res


def _neighbors_from_ranks(m1, m2, rank_lt):
    """Exact masked-min/max neighbor values from rank counts + sorted m2.

    Reproduces bit-for-bit
      l = min_j where(fl(m1[i]-m2[j]) <= EPS, BIG, fl(m1[i]-m2[j]))
      g = max_j where(fl(m1[i]-m2[j]) >= -EPS, -BIG, fl(m1[i]-m2[j]))
    with BIG/-BIG sentinels when no valid neighbor exists.
    """
    s = np.sort(m2)  # ascending
    epsf = np.float32(EPS)
    nepsf = np.float32(-EPS)
    bigf = np.float32(BIG)

    r_lt = rank_lt.astype(np.int64)
    r_le = r_lt + (np.searchsorted(s, m1, side="right")
                   - np.searchsorted(s, m1, side="left"))

    l = np.empty(N, np.float32)
    g = np.empty(N, np.float32)
    for i in range(N):
        q = np.float32(m1[i])
        # predecessor: largest m2 value with fl(q - v) > EPS
        k = min(r_lt[i] - 1, N - 1)
        while k + 1 < N and np.float32(q - s[k + 1]) > epsf:
            k += 1
        while k >= 0 and not (np.float32(q - s[k]) > epsf):
            k -= 1
        l[i] = np.float32(q - s[k]) if k >= 0 else bigf
        # successor: smallest m2 value with fl(q - v) < -EPS
        k = max(r_le[i], 0)
        while k - 1 >= 0 and np.float32(q - s[k - 1]) < nepsf:
            k -= 1
        while k < N and not (np.float32(q - s[k]) < nepsf):
            k += 1
        g[i] = np.float32(q - s[k]) if k < N else -bigf
    return l, g


def _epilogue(l, g):
    # the reference's O(N) tail, with the same eager jnp ops (the divide
    # rounding is backend-specific and IS the loss signal)
    import jax.numpy as jnp

    lj = jnp.asarray(l)
    gj = jnp.asarray(g)
    invalid = (lj == BIG) | (gj == -BIG)
    denom = gj + lj
    l_w = gj / denom
    g_w = lj / denom
    w = denom / (jnp.linalg.norm(denom) + EPS)
    grad_l = lj * l_w
    grad_g = gj * g_w
    loss_vec = jnp.abs(w * (grad_l - grad_g))
    valid = ~invalid
    n_valid = jnp.maximum(jnp.sum(valid.astype(jnp.float32)), 1.0)
    loss = jnp.sum(jnp.where(valid, loss_vec, 0.0)) / n_valid
    return np.asarray(loss)


def kernel(feats1, feats2):
    feats1 = np.asarray(feats1, dtype=np.float32)
    feats2 = np.asarray(feats2, dtype=np.float32)
    m1, m2, rank, _ = _run_device(feats1, feats2)
    l, g = _neighbors_from_ranks(m1, m2, rank)
    return _epilogue(l, g)


# revision 11
# speedup vs baseline: 1.0308x; 1.0308x over previous
"""Trainium2 Bass kernel for nn_BasisAlignment (loss_fn).

Math: the reference's [N,N] distance map factorizes as dm[i,j] = m1[i] - m2[j]
(m = row means). Its masked min/max over j are order selections:
  l_dist[i] = fl32(m1[i] - A[i]),  A[i] = max{ m2[j] : fl32(m1[i]-m2[j]) >  EPS }
  g_dist[i] = fl32(m1[i] - B[i]),  B[i] = min{ m2[j] : fl32(m1[i]-m2[j]) < -EPS }
so it suffices to know, per query m1[i], how many m2[j] lie strictly below it
(a rank count), plus the sorted m2 values.

Device (SPMD over 8 NeuronCores, feats row-sharded 512 rows/core):
  - row means of the local feats1/feats2 blocks: two sequential 512-wide DVE
    reduces + add + *2^-10 (bit-identical to jnp.mean's neuron lowering —
    verified on hardware)
  - AllGather of the m2 partial means -> full m2 [4096] on every core
  - rank counts: per local m1 query row, sum_j (m2[j] < m1[i]) via fused
    is_lt+accumulate DVE instructions (the hint's "local masked min/max", in
    count form; integer-valued fp32, exact, order-invariant). Computed in 4
    quarter passes pipelined behind partition-broadcast fills of m2.
Host: sort m2, convert ranks to neighbor values with an exact fp32 boundary
correction for the +-EPS band, then run the reference's tiny [N] epilogue with
the same eager jnp ops (their divide rounding is backend-specific and the
loss is at rounding-noise scale, so the epilogue must use identical ops).
"""
import numpy as np

N = 4096
D = 1024
NCORES = 8
RPC = N // NCORES        # rows per core
BPC = RPC // 128         # 128-row blocks per core
SEGS = (512, 1024, 1024, 1536)   # rank pass segment widths (sum = N)
NQ = len(SEGS)
NCOLS = 2 * BPC + NQ * BPC

BIG = 1.0e12
EPS = 1.0e-12

_compiled = None


def _build():
    import concourse.bacc as bacc
    import concourse.mybir as mybir
    from concourse import tile

    F32 = mybir.dt.float32
    AX = mybir.AxisListType
    OP = mybir.AluOpType

    nc = bacc.Bacc("TRN2", target_bir_lowering=False, debug=False,
                   num_devices=NCORES)
    f1 = nc.declare_dram_parameter("f1", [RPC, D], F32, isOutput=False)
    f2 = nc.declare_dram_parameter("f2", [RPC, D], F32, isOutput=False)
    # out columns: 0:4 m1 means, 4:8 m2 means, 8:24 rank quarter-counts
    # (col group 2+q, lane b): global row = core*512 + b*128 + partition
    out = nc.declare_dram_parameter("out", [128, NCOLS], F32, isOutput=True)

    with tile.TileContext(nc) as tc:
        with (
            tc.tile_pool(name="data", bufs=4 * BPC) as data_pool,
            tc.tile_pool(name="stage", bufs=1) as stage_pool,
            tc.tile_pool(name="wide", bufs=1) as wide_pool,
            tc.tile_pool(name="scr", bufs=2) as scr_pool,
            tc.tile_pool(name="dram", bufs=1, space="DRAM") as dram_pool,
        ):
            stage = stage_pool.tile([128, NCOLS], F32)
            m2_bounce = dram_pool.tile([128, BPC], F32)
            m2_all = dram_pool.tile([NCORES * 128, BPC], F32)

            def means(src, col0):
                # per 128-row block: half-tiles loaded separately so the
                # first reduce starts sooner; the reduce/add/scale structure
                # is bit-identical to jnp.mean's neuron lowering
                for b in range(BPC):
                    t0 = data_pool.tile([128, D // 2], F32, tag="th")
                    t1 = data_pool.tile([128, D // 2], F32, tag="th")
                    nc.sync.dma_start(t0[:], src[b * 128:(b + 1) * 128, 0:D // 2])
                    nc.sync.dma_start(t1[:], src[b * 128:(b + 1) * 128, D // 2:D])
                    r0 = scr_pool.tile([128, 1], F32, tag="r0")
                    r1 = scr_pool.tile([128, 1], F32, tag="r1")
                    nc.vector.tensor_reduce(r0[:], t0[:], axis=AX.X, op=OP.add)
                    nc.vector.tensor_reduce(r1[:], t1[:], axis=AX.X, op=OP.add)
                    h = scr_pool.tile([128, 1], F32, tag="h")
                    nc.vector.tensor_tensor(h[:], r0[:], r1[:], op=OP.add)
                    nc.vector.tensor_scalar_mul(
                        stage[:, col0 + b:col0 + b + 1], h[:], float(2.0 ** -10))

            # feats2 means first so the collective starts early
            means(f2, BPC)
            nc.sync.dma_start(m2_bounce[:, :], stage[:, BPC:2 * BPC])
            nc.gpsimd.collective_compute(
                "AllGather", OP.bypass,
                replica_groups=[list(range(NCORES))],
                ins=[m2_bounce.opt()], outs=[m2_all.opt()],
            )

            # scheduler fence: keep the f1 load dispatches behind the f2
            # chain + bounce so the collective isn't delayed by queued f1
            # transfers (f1 has the whole collective window to land)
            tc.no_sync_barrier()

            # feats1 means overlap with the collective
            means(f1, 0)

            # m2 broadcast to all partitions in segments pipelined against
            # the rank passes (counts are order-invariant, so the gathered
            # permutation of m2 is irrelevant). A small first segment gets
            # the first rank instruction started sooner after the
            # collective; the bigger tail segments amortize overhead while
            # the broadcasts stay ahead of the DVE.
            m2row = wide_pool.tile([1, N], F32, tag="m2row")
            flat = m2_all.rearrange("a b -> (a b)")
            m2b = wide_pool.tile([128, N], F32, tag="m2b")
            off = 0
            for q, w in enumerate(SEGS):
                nc.sync.dma_start(m2row[:, off:off + w], flat[off:off + w])
                nc.gpsimd.partition_broadcast(
                    m2b[:, off:off + w], m2row[:, off:off + w])
                for b in range(BPC):
                    junk = scr_pool.tile([128, max(SEGS)], F32, tag="junk")
                    nc.vector.tensor_scalar(
                        out=junk[:, :w], in0=m2b[:, off:off + w],
                        scalar1=stage[:, b:b + 1], scalar2=0.0,
                        op0=OP.is_lt, op1=OP.add,
                        accum_out=stage[:, (2 + q) * BPC + b:(2 + q) * BPC + b + 1],
                    )
                off += w

            nc.sync.dma_start(out[:, :], stage[:])

    nc.compile()
    return nc


def _get_compiled():
    global _compiled
    if _compiled is None:
        _compiled = _build()
    return _compiled


def _run_device(feats1, feats2, trace=False):
    from concourse.bass_utils import run_bass_kernel_spmd

    nc = _get_compiled()
    in_maps = []
    for c in range(NCORES):
        in_maps.append({
            "f1": np.ascontiguousarray(feats1[c * RPC:(c + 1) * RPC], dtype=np.float32),
            "f2": np.ascontiguousarray(feats2[c * RPC:(c + 1) * RPC], dtype=np.float32),
        })
    res = run_bass_kernel_spmd(nc, in_maps, list(range(NCORES)), trace=trace)
    m1 = np.empty(N, np.float32)
    m2 = np.empty(N, np.float32)
    rank = np.zeros(N, np.float64)
    for c in range(NCORES):
        o = res.results[c]["out"]  # [128, NCOLS]
        for b in range(BPC):
            sl = slice(c * RPC + b * 128, c * RPC + (b + 1) * 128)
            m1[sl] = o[:, b]
            m2[sl] = o[:, BPC + b]
            acc = np.zeros(128, np.float64)
            for q in range(NQ):
                acc += o[:, (2 + q) * BPC + b].astype(np.float64)
            rank[sl] = acc
    return m1, m2, rank, res


def _neighbors_from_ranks(m1, m2, rank_lt):
    """Exact masked-min/max neighbor values from rank counts + sorted m2.

    Reproduces bit-for-bit
      l = min_j where(fl(m1[i]-m2[j]) <= EPS, BIG, fl(m1[i]-m2[j]))
      g = max_j where(fl(m1[i]-m2[j]) >= -EPS, -BIG, fl(m1[i]-m2[j]))
    with BIG/-BIG sentinels when no valid neighbor exists.
    """
    s = np.sort(m2)  # ascending
    epsf = np.float32(EPS)
    nepsf = np.float32(-EPS)
    bigf = np.float32(BIG)

    r_lt = rank_lt.astype(np.int64)
    r_le = r_lt + (np.searchsorted(s, m1, side="right")
                   - np.searchsorted(s, m1, side="left"))

    l = np.empty(N, np.float32)
    g = np.empty(N, np.float32)
    for i in range(N):
        q = np.float32(m1[i])
        # predecessor: largest m2 value with fl(q - v) > EPS
        k = min(r_lt[i] - 1, N - 1)
        while k + 1 < N and np.float32(q - s[k + 1]) > epsf:
            k += 1
        while k >= 0 and not (np.float32(q - s[k]) > epsf):
            k -= 1
        l[i] = np.float32(q - s[k]) if k >= 0 else bigf
        # successor: smallest m2 value with fl(q - v) < -EPS
        k = max(r_le[i], 0)
        while k - 1 >= 0 and np.float32(q - s[k - 1]) < nepsf:
            k -= 1
        while k < N and not (np.float32(q - s[k]) < nepsf):
            k += 1
        g[i] = np.float32(q - s[k]) if k < N else -bigf
    return l, g


def _epilogue(l, g):
    # the reference's O(N) tail, with the same eager jnp ops (the divide
    # rounding is backend-specific and IS the loss signal)
    import jax.numpy as jnp

    lj = jnp.asarray(l)
    gj = jnp.asarray(g)
    invalid = (lj == BIG) | (gj == -BIG)
    denom = gj + lj
    l_w = gj / denom
    g_w = lj / denom
    w = denom / (jnp.linalg.norm(denom) + EPS)
    grad_l = lj * l_w
    grad_g = gj * g_w
    loss_vec = jnp.abs(w * (grad_l - grad_g))
    valid = ~invalid
    n_valid = jnp.maximum(jnp.sum(valid.astype(jnp.float32)), 1.0)
    loss = jnp.sum(jnp.where(valid, loss_vec, 0.0)) / n_valid
    return np.asarray(loss)


def kernel(feats1, feats2):
    feats1 = np.asarray(feats1, dtype=np.float32)
    feats2 = np.asarray(feats2, dtype=np.float32)
    m1, m2, rank, _ = _run_device(feats1, feats2)
    l, g = _neighbors_from_ranks(m1, m2, rank)
    return _epilogue(l, g)


# revision 16
# speedup vs baseline: 1.0381x; 1.0071x over previous
"""Trainium2 Bass kernel for nn_BasisAlignment (loss_fn).

Math: the reference's [N,N] distance map factorizes as dm[i,j] = m1[i] - m2[j]
(m = row means). Its masked min/max over j are order selections:
  l_dist[i] = fl32(m1[i] - A[i]),  A[i] = max{ m2[j] : fl32(m1[i]-m2[j]) >  EPS }
  g_dist[i] = fl32(m1[i] - B[i]),  B[i] = min{ m2[j] : fl32(m1[i]-m2[j]) < -EPS }
so it suffices to know, per query m1[i], how many m2[j] lie strictly below it
(a rank count), plus the sorted m2 values.

Device (SPMD over 8 NeuronCores, feats row-sharded 512 rows/core):
  - row means of the local feats1/feats2 blocks: two sequential 512-wide DVE
    reduces + add + *2^-10 (bit-identical to jnp.mean's neuron lowering —
    verified on hardware)
  - AllGather of the m2 partial means -> full m2 [4096] on every core
  - rank counts: per local m1 query row, sum_j (m2[j] < m1[i]) via fused
    is_lt+accumulate DVE instructions (the hint's "local masked min/max", in
    count form; integer-valued fp32, exact, order-invariant). Computed in 4
    quarter passes pipelined behind partition-broadcast fills of m2.
Host: sort m2, convert ranks to neighbor values with an exact fp32 boundary
correction for the +-EPS band, then run the reference's tiny [N] epilogue with
the same eager jnp ops (their divide rounding is backend-specific and the
loss is at rounding-noise scale, so the epilogue must use identical ops).
"""
import numpy as np

N = 4096
D = 1024
NCORES = 8
RPC = N // NCORES        # rows per core
BPC = RPC // 128         # 128-row blocks per core
SEGS = (512, 768, 1280, 1536)    # rank pass segment widths (sum = N)
NQ = len(SEGS)
NCOLS = 2 * BPC + NQ * BPC

BIG = 1.0e12
EPS = 1.0e-12

_compiled = None


def _build():
    import concourse.bacc as bacc
    import concourse.mybir as mybir
    from concourse import tile

    F32 = mybir.dt.float32
    AX = mybir.AxisListType
    OP = mybir.AluOpType

    nc = bacc.Bacc("TRN2", target_bir_lowering=False, debug=False,
                   num_devices=NCORES)
    f1 = nc.declare_dram_parameter("f1", [RPC, D], F32, isOutput=False)
    f2 = nc.declare_dram_parameter("f2", [RPC, D], F32, isOutput=False)
    # out columns: 0:4 m1 means, 4:8 m2 means, 8:24 rank quarter-counts
    # (col group 2+q, lane b): global row = core*512 + b*128 + partition
    out = nc.declare_dram_parameter("out", [128, NCOLS], F32, isOutput=True)

    with tile.TileContext(nc) as tc:
        with (
            tc.tile_pool(name="data", bufs=4 * BPC) as data_pool,
            tc.tile_pool(name="stage", bufs=1) as stage_pool,
            tc.tile_pool(name="wide", bufs=1) as wide_pool,
            tc.tile_pool(name="scr", bufs=2) as scr_pool,
            tc.tile_pool(name="dram", bufs=1, space="DRAM") as dram_pool,
        ):
            stage = stage_pool.tile([128, NCOLS], F32)
            m2_bounce = dram_pool.tile([128, BPC], F32)
            m2_all = dram_pool.tile([NCORES * 128, BPC], F32)

            def means(src, col0):
                # per 128-row block: half-tiles loaded separately so the
                # first reduce starts sooner. The two sequential 512-wide
                # reduces + add reproduce jnp.mean's neuron lowering up to
                # the final *2^-10, which is EXACT in fp32 and therefore
                # order-preserving (ties included) — the rank comparisons
                # run on raw sums and the host applies the exact scaling.
                for b in range(BPC):
                    t0 = data_pool.tile([128, D // 2], F32, tag="th")
                    t1 = data_pool.tile([128, D // 2], F32, tag="th")
                    nc.sync.dma_start(t0[:], src[b * 128:(b + 1) * 128, 0:D // 2])
                    nc.sync.dma_start(t1[:], src[b * 128:(b + 1) * 128, D // 2:D])
                    r0 = scr_pool.tile([128, 1], F32, tag="r0")
                    r1 = scr_pool.tile([128, 1], F32, tag="r1")
                    nc.vector.tensor_reduce(r0[:], t0[:], axis=AX.X, op=OP.add)
                    nc.vector.tensor_reduce(r1[:], t1[:], axis=AX.X, op=OP.add)
                    nc.vector.tensor_tensor(
                        stage[:, col0 + b:col0 + b + 1], r0[:], r1[:], op=OP.add)

            # feats2 means first so the collective starts early
            means(f2, BPC)
            nc.sync.dma_start(m2_bounce[:, :], stage[:, BPC:2 * BPC])
            nc.gpsimd.collective_compute(
                "AllGather", OP.bypass,
                replica_groups=[list(range(NCORES))],
                ins=[m2_bounce.opt()], outs=[m2_all.opt()],
            )

            # scheduler fence: keep the f1 load dispatches behind the f2
            # chain + bounce so the collective isn't delayed by queued f1
            # transfers (f1 has the whole collective window to land)
            tc.no_sync_barrier()

            # feats1 means overlap with the collective; their columns DMA
            # out early so the final out transfer is rank-columns only
            means(f1, 0)
            nc.sync.dma_start(out[:, 0:2 * BPC], stage[:, 0:2 * BPC])

            # m2 broadcast to all partitions in segments pipelined against
            # the rank passes (counts are order-invariant, so the gathered
            # permutation of m2 is irrelevant). A small first segment gets
            # the first rank instruction started sooner after the
            # collective; the bigger tail segments amortize overhead while
            # the broadcasts stay ahead of the DVE.
            m2row = wide_pool.tile([1, N], F32, tag="m2row")
            flat = m2_all.rearrange("a b -> (a b)")
            m2b = wide_pool.tile([128, N], F32, tag="m2b")
            off = 0
            for q, w in enumerate(SEGS):
                nc.sync.dma_start(m2row[:, off:off + w], flat[off:off + w])
                nc.gpsimd.partition_broadcast(
                    m2b[:, off:off + w], m2row[:, off:off + w])
                for b in range(BPC):
                    junk = scr_pool.tile([128, max(SEGS)], F32, tag="junk")
                    nc.vector.tensor_scalar(
                        out=junk[:, :w], in0=m2b[:, off:off + w],
                        scalar1=stage[:, b:b + 1], scalar2=0.0,
                        op0=OP.is_lt, op1=OP.add,
                        accum_out=stage[:, (2 + q) * BPC + b:(2 + q) * BPC + b + 1],
                    )
                off += w

            nc.sync.dma_start(out[:, 2 * BPC:NCOLS], stage[:, 2 * BPC:NCOLS])

    nc.compile()
    return nc


def _get_compiled():
    global _compiled
    if _compiled is None:
        _compiled = _build()
    return _compiled


def _run_device(feats1, feats2, trace=False):
    from concourse.bass_utils import run_bass_kernel_spmd

    nc = _get_compiled()
    in_maps = []
    for c in range(NCORES):
        in_maps.append({
            "f1": np.ascontiguousarray(feats1[c * RPC:(c + 1) * RPC], dtype=np.float32),
            "f2": np.ascontiguousarray(feats2[c * RPC:(c + 1) * RPC], dtype=np.float32),
        })
    res = run_bass_kernel_spmd(nc, in_maps, list(range(NCORES)), trace=trace)
    m1 = np.empty(N, np.float32)
    m2 = np.empty(N, np.float32)
    rank = np.zeros(N, np.float64)
    scale = np.float32(2.0 ** -10)   # exact in fp32: sum*2^-10 == jnp.mean bits
    for c in range(NCORES):
        o = res.results[c]["out"]  # [128, NCOLS]; mean cols hold raw sums
        for b in range(BPC):
            sl = slice(c * RPC + b * 128, c * RPC + (b + 1) * 128)
            m1[sl] = o[:, b] * scale
            m2[sl] = o[:, BPC + b] * scale
            acc = np.zeros(128, np.float64)
            for q in range(NQ):
                acc += o[:, (2 + q) * BPC + b].astype(np.float64)
            rank[sl] = acc
    return m1, m2, rank, res


def _neighbors_from_ranks(m1, m2, rank_lt):
    """Exact masked-min/max neighbor values from rank counts + sorted m2.

    Reproduces bit-for-bit
      l = min_j where(fl(m1[i]-m2[j]) <= EPS, BIG, fl(m1[i]-m2[j]))
      g = max_j where(fl(m1[i]-m2[j]) >= -EPS, -BIG, fl(m1[i]-m2[j]))
    with BIG/-BIG sentinels when no valid neighbor exists.
    """
    s = np.sort(m2)  # ascending
    epsf = np.float32(EPS)
    nepsf = np.float32(-EPS)
    bigf = np.float32(BIG)

    r_lt = rank_lt.astype(np.int64)
    r_le = r_lt + (np.searchsorted(s, m1, side="right")
                   - np.searchsorted(s, m1, side="left"))

    l = np.empty(N, np.float32)
    g = np.empty(N, np.float32)
    for i in range(N):
        q = np.float32(m1[i])
        # predecessor: largest m2 value with fl(q - v) > EPS
        k = min(r_lt[i] - 1, N - 1)
        while k + 1 < N and np.float32(q - s[k + 1]) > epsf:
            k += 1
        while k >= 0 and not (np.float32(q - s[k]) > epsf):
            k -= 1
        l[i] = np.float32(q - s[k]) if k >= 0 else bigf
        # successor: smallest m2 value with fl(q - v) < -EPS
        k = max(r_le[i], 0)
        while k - 1 >= 0 and np.float32(q - s[k - 1]) < nepsf:
            k -= 1
        while k < N and not (np.float32(q - s[k]) < nepsf):
            k += 1
        g[i] = np.float32(q - s[k]) if k < N else -bigf
    return l, g


def _epilogue(l, g):
    # the reference's O(N) tail, with the same eager jnp ops (the divide
    # rounding is backend-specific and IS the loss signal)
    import jax.numpy as jnp

    lj = jnp.asarray(l)
    gj = jnp.asarray(g)
    invalid = (lj == BIG) | (gj == -BIG)
    denom = gj + lj
    l_w = gj / denom
    g_w = lj / denom
    w = denom / (jnp.linalg.norm(denom) + EPS)
    grad_l = lj * l_w
    grad_g = gj * g_w
    loss_vec = jnp.abs(w * (grad_l - grad_g))
    valid = ~invalid
    n_valid = jnp.maximum(jnp.sum(valid.astype(jnp.float32)), 1.0)
    loss = jnp.sum(jnp.where(valid, loss_vec, 0.0)) / n_valid
    return np.asarray(loss)


def kernel(feats1, feats2):
    feats1 = np.asarray(feats1, dtype=np.float32)
    feats2 = np.asarray(feats2, dtype=np.float32)
    m1, m2, rank, _ = _run_device(feats1, feats2)
    l, g = _neighbors_from_ranks(m1, m2, rank)
    return _epilogue(l, g)
